# revision 20
# baseline (speedup 1.0000x reference)
"""MoE model (embed -> gate -> 4 dense experts -> softmax combine) on 8 TRN2 cores.

Key algebraic restructuring vs the naive dense pipeline: the tokens only index
V=512 distinct embedding rows per table, so the entire first-layer expert
matmul (e @ W1, 8.4 MMAC/token — 95% of the model's FLOPs) is precomputed
per *vocab entry* instead of per token:

  A0[v] = concat_e(emb0[v] @ W1[e,:1024]) (+b1)   -> [V, E*D (+gating col)]
  A1[v] = concat_e(emb1[v] @ W1[e,1024:])         -> [V, E*D (+gating col)]
  z[t]  = A0[x0[t]] + A1[x1[t]]                   (gather + add)
  out[t]= sum_e softmax_e(z_gate) * (W2[e] @ silu(z[t,e,:]) + b2)

The gating logits (e @ Wg + bg) are folded into the same tables as a 33rd
128-wide column chunk, so one gather feeds both the experts and the gate.

Per core (8192 tokens): the A tables (4.2 MB each, bf16) are built on the PE
at kernel start (fm matmul + PE transpose) and written to DRAM scratch; the
main loop gathers token rows with *non-transposing* gpsimd gather DMAs
(token-major, 1 descriptor/row — descriptor-gen stays off the critical path),
adds them on the DVE, transposes z back to feature-major on the PE (128x128
identity matmuls into PSUM), applies Silu on the scalar engine straight out
of PSUM, and runs the small W2 matmul + softmax-weighted combine as before.

bf16 tables with fp32 PSUM accumulation: rel err vs fp32 reference ~0.5%.
"""

import os
import numpy as np
import ml_dtypes

import concourse.bass as bass
import concourse.mybir as mybir
import concourse.tile as tile
from concourse.bass_utils import run_bass_kernel_spmd

BF16 = ml_dtypes.bfloat16

B = 65536
V = 512
D = 1024
IN = 2048
E = 4
OUT = 128
NCORES = 8
BL = B // NCORES          # tokens per core
ST = 256                  # tokens per supertile
NST = BL // ST            # supertiles per core
KC8 = 8                   # 128-chunks of one table-half's input dim (1024)
JE = 32                   # expert-feature chunks per table row (E*D/128)
JW = JE + 1               # + 1 gating chunk
ROWE = JW * 128           # table row length in elements (4224)
DC = D // 128

LAST_EXEC_NS = None       # set when BASSMOE_TRACE=1
LAST_RES = None


def _legalize_waits(nc, max_waits=1):
    """This walrus build rejects instructions carrying more than ~1 sync-wait
    command ("Too many sync wait commands", CoreV2/V3GenImpl setupSyncWait).
    Hoist all but the last wait of every instruction onto single-wait NoOps
    placed immediately before it in the same engine's stream."""
    for f in nc.m.functions:
        for bb in f.blocks:
            insts = bb.instructions
            if not any(
                inst.sync_info is not None and len(inst.sync_info.on_wait) > max_waits
                for inst in insts
            ):
                continue
            new = []
            for inst in insts:
                si = inst.sync_info
                waits = list(si.on_wait) if si is not None else []
                if len(waits) > max_waits:
                    for w in waits[:-max_waits]:
                        nop = mybir.InstNoOp(
                            name=f"legw-{nc.next_id()}", ins=[], outs=[]
                        )
                        nop.engine = inst.engine
                        nop.sync_info = mybir.SyncInfo(on_wait=[w], on_update=[])
                        new.append(nop)
                    inst.sync_info = mybir.SyncInfo(
                        on_wait=waits[-max_waits:], on_update=list(si.on_update)
                    )
                new.append(inst)
            bb.instructions = new


def _build_tables(nc, tc, ALU, f32, bf16, embtd, bias0d, w1d, atd, ident_sb):
    with (
        tc.tile_pool(name="procst", bufs=1) as procst,
        tc.tile_pool(name="w1st", bufs=4) as w1pool,
        tc.tile_pool(name="stg", bufs=3) as stpool,
        tc.tile_pool(name="rowt", bufs=1) as rowpool,
        tc.tile_pool(name="ppa", bufs=1, space="PSUM") as ppa,
        tc.tile_pool(name="ppt", bufs=1, space="PSUM") as ppt,
    ):
        bias0_sb = procst.tile([128, JW], f32)
        nc.sync.dma_start(bias0_sb[:], bias0d[:])

        for t in range(2):
            embt_sb = procst.tile([128, KC8, V], bf16, tag="embt")
            nc.sync.dma_start(embt_sb[:], embtd[:, t, :, :])
            rows_all = rowpool.tile([128, V // 128, ROWE], bf16, tag="rows")
            for c in range(JW):
                w1c = w1pool.tile([128, KC8, 128], bf16, tag="w1c")
                nc.sync.dma_start(w1c[:], w1d[t, c])
                psA = ppa.tile([128, V], f32, tag="pa")
                for kc in range(KC8):
                    nc.tensor.matmul(
                        psA[:],
                        w1c[:, kc, :],
                        embt_sb[:, kc, :],
                        start=(kc == 0),
                        stop=(kc == KC8 - 1),
                    )
                stage = stpool.tile([128, V], bf16, tag="stg")
                if t == 0:
                    nc.vector.tensor_scalar(
                        stage[:], psA[:], bias0_sb[:, c : c + 1], None, ALU.add
                    )
                else:
                    nc.vector.tensor_copy(stage[:], psA[:])
                psT = ppt.tile([128, V // 128, 128], bf16, tag="pt")
                for vc in range(V // 128):
                    nc.tensor.transpose(
                        psT[:, vc, :],
                        stage[:, vc * 128 : (vc + 1) * 128],
                        ident_sb[:],
                    )
                nc.scalar.copy(
                    rows_all[:, :, c * 128 : (c + 1) * 128], psT[:]
                )
            for vc in range(V // 128):
                nc.sync.dma_start(
                    atd[t * V + vc * 128 : t * V + (vc + 1) * 128, :],
                    rows_all[:, vc, :],
                )


def build_program(nst=NST, legalize=True):
    dt = mybir.dt
    f32, bf16 = dt.float32, dt.bfloat16
    AF = mybir.ActivationFunctionType
    ALU = mybir.AluOpType

    nc = bass.Bass()

    xd = nc.dram_tensor("xi01", [128, nst, 2 * ST // 16], dt.int16, kind="ExternalInput")
    embtd = nc.dram_tensor("embt", [128, 2, KC8, V], bf16, kind="ExternalInput")
    w1d = nc.dram_tensor("w1s", [2, JW, 128, KC8, 128], bf16, kind="ExternalInput")
    w2d = nc.dram_tensor("w2s", [128, E, DC, OUT], bf16, kind="ExternalInput")
    b2d = nc.dram_tensor("b2s", [128, E], f32, kind="ExternalInput")
    bias0d = nc.dram_tensor("bias0", [128, JW], f32, kind="ExternalInput")
    identd = nc.dram_tensor("ident", [128, 128], bf16, kind="ExternalInput")
    seld = nc.dram_tensor("sels", [E, E, 128], bf16, kind="ExternalInput")
    outd = nc.dram_tensor("out", [128, nst * ST], f32, kind="ExternalOutput")

    # combined A table (both halves stacked), DRAM scratch
    atd = nc.dram_tensor("at01", [2 * V, ROWE], bf16, kind="Internal")

    with tile.TileContext(nc) as tc:
        with (
            tc.tile_pool(name="const", bufs=1) as cpool,
            tc.tile_pool(name="ga", bufs=3) as apool,
            tc.tile_pool(name="zt", bufs=1) as zpool,
            tc.tile_pool(name="ht", bufs=1) as hpool,
            tc.tile_pool(name="sm", bufs=2) as smpool,
            tc.tile_pool(name="gsb", bufs=2) as gspool,
            tc.tile_pool(name="outp", bufs=2) as opool,
            tc.tile_pool(name="pzs", bufs=2, space="PSUM") as pzs,
            tc.tile_pool(name="peo", bufs=2, space="PSUM") as peo,
            tc.tile_pool(name="pgp", bufs=1, space="PSUM") as pgp,
            tc.tile_pool(name="pms", bufs=1, space="PSUM") as pms,
        ):
            from concourse import library_config

            nc.gpsimd.load_library(library_config.mlp)
            streg = nc.gpsimd.to_reg(2 * ST)

            # --- resident constants ---
            ident_sb = cpool.tile([128, 128], bf16)
            nc.sync.dma_start(ident_sb[:], identd[:])
            w2_sb = cpool.tile([128, E, DC, OUT], bf16)
            nc.sync.dma_start(w2_sb[:], w2d[:])
            b2_sb = cpool.tile([128, E], f32)
            nc.sync.dma_start(b2_sb[:], b2d[:])
            sel_sb = cpool.tile([E, E, 128], bf16)
            nc.sync.dma_start(sel_sb[:], seld[:])
            ones4 = cpool.tile([E, 1], bf16)
            nc.vector.memset(ones4[:], 1.0)
            ones14 = cpool.tile([1, E], bf16)
            nc.vector.memset(ones14[:], 1.0)

            # --- prologue: build A tables on the PE, write to DRAM scratch ---
            _build_tables(
                nc, tc, ALU, f32, bf16, embtd, bias0d, w1d, atd, ident_sb
            )

            # --- all gather indices resident up front (one DMA) ---
            xi_all = cpool.tile([128, nst, 2 * ST // 16], dt.int16, tag="xia")
            nc.sync.dma_start(xi_all[:], xd[:])

            # --- main loop: gather -> add -> transpose -> silu -> W2 -> mix ---
            def issue_gather(i):
                at_ = apool.tile([128, 2 * ST // 128, ROWE], bf16, tag="a")
                nc.gpsimd.dma_gather(
                    out_ap=at_[:],
                    in_ap=atd[:],
                    idxs_ap=xi_all[:, i, :],
                    num_idxs=2 * ST,
                    num_idxs_reg=streg,
                    elem_size=ROWE,
                    transpose=False,
                )
                return at_

            pend = [issue_gather(0)]
            if nst > 1:
                pend.append(issue_gather(1))

            NG = ST // 128
            for i in range(nst):
                at_ = pend.pop(0)
                z = zpool.tile([128, NG, ROWE], bf16, tag="z")
                for q in range(4):
                    c0 = q * 1024 if q < 3 else 3072
                    c1 = (q + 1) * 1024 if q < 3 else ROWE
                    nc.vector.tensor_tensor(
                        z[:, :, c0:c1],
                        at_[:, 0:NG, c0:c1],
                        at_[:, NG : 2 * NG, c0:c1],
                        ALU.add,
                    )

                if i + 2 < nst:
                    pend.append(issue_gather(i + 2))

                h = hpool.tile([128, JE, ST], bf16, tag="h")
                for k in range(JE // 4):
                    zp = pzs.tile([128, 4, ST], bf16, tag="zp")
                    for j in range(4):
                        fc = 4 * k + j
                        for g in range(ST // 128):
                            nc.tensor.transpose(
                                zp[:, j, g * 128 : (g + 1) * 128],
                                z[:, g, fc * 128 : (fc + 1) * 128],
                                ident_sb[:],
                            )
                    nc.scalar.activation(h[:, 4 * k : 4 * k + 4, :], zp[:], AF.Silu)

                # gating chunk (feature-major logits on partitions 0..3)
                pg = pzs.tile([128, 4, ST], bf16, tag="zp")
                for g in range(ST // 128):
                    nc.tensor.transpose(
                        pg[:, 0, g * 128 : (g + 1) * 128],
                        z[:, g, JE * 128 : JE * 128 + 128],
                        ident_sb[:],
                    )
                expt = smpool.tile([E, ST], bf16, tag="expt")
                nc.scalar.activation(expt[:], pg[0:E, 0, :], AF.Exp)
                gsum = pms.tile([128, ST], f32, tag="gsum")
                sp = gsum[0:1, :]
                nc.tensor.matmul(sp, ones4[:], expt[:], start=True, stop=True)
                rec = smpool.tile([1, ST], f32, tag="rec")
                nc.vector.reciprocal_approx_fast(rec[:], sp)
                recb = smpool.tile([1, ST], bf16, tag="recb")
                nc.vector.tensor_copy(recb[:], rec[:])
                rb4 = gsum[32:36, :]
                nc.tensor.matmul(rb4, ones14[:], recb[:], start=True, stop=True)
                gates = smpool.tile([E, ST], bf16, tag="gates")
                nc.vector.tensor_tensor(gates[:], expt[:], rb4, ALU.mult)

                acc = opool.tile([128, ST], f32, tag="acc")
                for e in range(E):
                    eop = peo.tile([128, ST], f32, tag="eo")
                    for dc in range(DC):
                        nc.tensor.matmul(
                            eop[:],
                            w2_sb[:, e, dc, :],
                            h[:, e * DC + dc, :],
                            start=(dc == 0),
                            stop=(dc == DC - 1),
                        )
                    gp = pgp.tile([128, ST], f32, tag="gp")
                    nc.tensor.matmul(
                        gp[:], sel_sb[:, e, :], gates[:], start=True, stop=True
                    )
                    gpsb = gspool.tile([128, ST], bf16, tag="gpsb")
                    nc.vector.tensor_copy(gpsb[:], gp[:])
                    if e == 0:
                        nc.vector.scalar_tensor_tensor(
                            acc[:], eop[:], b2_sb[:, e : e + 1], gpsb[:],
                            ALU.add, ALU.mult,
                        )
                    else:
                        tmp = opool.tile([128, ST], f32, tag="tmp")
                        nc.vector.scalar_tensor_tensor(
                            tmp[:], eop[:], b2_sb[:, e : e + 1], gpsb[:],
                            ALU.add, ALU.mult,
                        )
                        nc.vector.tensor_add(acc[:], acc[:], tmp[:])
                nc.sync.dma_start(outd[:, i * ST : (i + 1) * ST], acc[:])

    if legalize:
        _legalize_waits(nc)
    mybir.codegen_inst_isa_subclasses(nc)
    return nc


def marshal_inputs(x, emb0, emb1, W1, b1, W2, b2, Wg, bg, nst=NST, ncores=NCORES):
    """Host-side: cast/reshape full inputs into per-core in_maps."""
    n_tok = ncores * nst * ST

    x = np.asarray(x)
    idx = np.concatenate(
        [
            x[:n_tok, 0].reshape(ncores, nst, ST),
            x[:n_tok, 1].reshape(ncores, nst, ST) + V,
        ],
        axis=2,
    ).astype(np.int16)                         # [c, nst, 2*ST]
    w = idx.reshape(ncores, nst, 2 * ST // 16, 16).transpose(0, 1, 3, 2)
    w = np.tile(w, (1, 1, 8, 1))               # [c, nst, 128, 2*ST//16]
    xh = {"xi01": np.ascontiguousarray(w.transpose(0, 2, 1, 3))}

    shared = {}
    # embt[p, t, kc, v] = emb_t[v, kc*128+p]  (partition-major, matches tile)
    embt = np.stack(
        [
            np.asarray(e).T.reshape(KC8, 128, V).transpose(1, 0, 2)
            for e in (emb0, emb1)
        ]
    ).transpose(1, 0, 2, 3)
    shared["embt"] = np.ascontiguousarray(embt.astype(BF16))

    # w1s[t, c, p, kc, j]: c<32 -> W1[e=c//8][t*1024+kc*128+p, (c%8)*128+j]
    #                      c=32 -> Wg[t*1024+kc*128+p, j] for j<4 else 0
    W1r = np.asarray(W1).reshape(E, 2, KC8, 128, DC, 128)
    w1e = np.transpose(W1r, (1, 0, 4, 3, 2, 5)).reshape(2, JE, 128, KC8, 128)
    wgt = np.zeros((2, 1, 128, KC8, 128), dtype=np.float32)
    Wgr = np.asarray(Wg).reshape(2, KC8, 128, E)
    wgt[:, 0, :, :, :E] = Wgr.transpose(0, 2, 1, 3)
    shared["w1s"] = np.ascontiguousarray(
        np.concatenate([w1e, wgt], axis=1).astype(BF16)
    )

    shared["w2s"] = np.ascontiguousarray(
        np.asarray(W2).reshape(E, DC, 128, OUT).transpose(2, 0, 1, 3).astype(BF16)
    )
    shared["b2s"] = np.ascontiguousarray(np.asarray(b2).T.astype(np.float32))

    bias0 = np.zeros((128, JW), dtype=np.float32)
    bias0[:, :JE] = np.asarray(b1).reshape(E, DC, 128).transpose(2, 0, 1).reshape(128, JE)
    bias0[:E, JE] = np.asarray(bg)
    shared["bias0"] = np.ascontiguousarray(bias0)

    shared["ident"] = np.ascontiguousarray(np.eye(128, dtype=np.float32).astype(BF16))
    shared["sels"] = np.ascontiguousarray(
        np.broadcast_to(np.eye(E, dtype=np.float32)[:, :, None], (E, E, 128)).astype(
            BF16
        )
    )
    return [{**{k: v[c] for k, v in xh.items()}, **shared} for c in range(ncores)]


def kernel(x, emb0, emb1, W1, b1, W2, b2, Wg, bg):
    global LAST_EXEC_NS, LAST_RES
    nc = build_program()
    in_maps = marshal_inputs(x, emb0, emb1, W1, b1, W2, b2, Wg, bg)
    trace = os.environ.get("BASSMOE_TRACE", "0") == "1"
    res = run_bass_kernel_spmd(nc, in_maps, list(range(NCORES)), trace=trace)
    LAST_EXEC_NS = res.exec_time_ns
    LAST_RES = res
    out = np.empty((B, OUT), dtype=np.float32)
    for c in range(NCORES):
        out[c * BL : (c + 1) * BL, :] = res.results[c]["out"].T
    return out


# revision 24
# speedup vs baseline: 1.1026x; 1.1026x over previous
"""MoE model (embed -> gate -> 4 dense experts -> softmax combine) on 8 TRN2 cores.

Key algebraic restructuring vs the naive dense pipeline: the tokens only index
V=512 distinct embedding rows per table, so the entire first-layer expert
matmul (e @ W1, 8.4 MMAC/token — 95% of the model's FLOPs) is precomputed
per *vocab entry* instead of per token:

  A0[v] = concat_e(emb0[v] @ W1[e,:1024]) (+b1)   -> [V, E*D (+gating col)]
  A1[v] = concat_e(emb1[v] @ W1[e,1024:])         -> [V, E*D (+gating col)]
  z[t]  = A0[x0[t]] + A1[x1[t]]                   (gather + add)
  out[t]= sum_e softmax_e(z_gate) * (W2[e] @ silu(z[t,e,:]) + b2)

The gating logits (e @ Wg + bg) are folded into the same tables as a 33rd
128-wide column chunk, so one gather feeds both the experts and the gate.

Per core (8192 tokens): the A tables (4.2 MB each, bf16) are built on the PE
at kernel start (fm matmul + PE transpose) and written to DRAM scratch; the
main loop gathers token rows with *non-transposing* gpsimd gather DMAs
(token-major, 1 descriptor/row — descriptor-gen stays off the critical path),
adds them on the DVE, transposes z back to feature-major on the PE (128x128
identity matmuls into PSUM), applies Silu on the scalar engine straight out
of PSUM, and runs the small W2 matmul + softmax-weighted combine as before.

bf16 tables with fp32 PSUM accumulation: rel err vs fp32 reference ~0.5%.
"""

import os
import numpy as np
import ml_dtypes

import concourse.bass as bass
import concourse.mybir as mybir
import concourse.tile as tile
from concourse.bass_utils import run_bass_kernel_spmd

BF16 = ml_dtypes.bfloat16

B = 65536
V = 512
D = 1024
IN = 2048
E = 4
OUT = 128
NCORES = 8
BL = B // NCORES          # tokens per core
ST = 256                  # tokens per supertile
NST = BL // ST            # supertiles per core
KC8 = 8                   # 128-chunks of one table-half's input dim (1024)
JE = 32                   # expert-feature chunks per table row (E*D/128)
JW = JE + 1               # + 1 gating chunk
ROWE = JW * 128           # table row length in elements (4224)
DC = D // 128

LAST_EXEC_NS = None       # set when BASSMOE_TRACE=1
LAST_RES = None


def _legalize_waits(nc, max_waits=1):
    """This walrus build rejects instructions carrying more than ~1 sync-wait
    command ("Too many sync wait commands", CoreV2/V3GenImpl setupSyncWait).
    Hoist all but the last wait of every instruction onto single-wait NoOps
    placed immediately before it in the same engine's stream."""
    for f in nc.m.functions:
        for bb in f.blocks:
            insts = bb.instructions
            if not any(
                inst.sync_info is not None and len(inst.sync_info.on_wait) > max_waits
                for inst in insts
            ):
                continue
            new = []
            for inst in insts:
                si = inst.sync_info
                waits = list(si.on_wait) if si is not None else []
                if len(waits) > max_waits:
                    for w in waits[:-max_waits]:
                        nop = mybir.InstNoOp(
                            name=f"legw-{nc.next_id()}", ins=[], outs=[]
                        )
                        nop.engine = inst.engine
                        nop.sync_info = mybir.SyncInfo(on_wait=[w], on_update=[])
                        new.append(nop)
                    inst.sync_info = mybir.SyncInfo(
                        on_wait=waits[-max_waits:], on_update=list(si.on_update)
                    )
                new.append(inst)
            bb.instructions = new


def _build_tables(nc, tc, ALU, f32, bf16, embtd, bias0d, w1d, wgd, atd, ident_sb, apool):
    with (
        tc.tile_pool(name="procst", bufs=1) as procst,
        tc.tile_pool(name="w1st", bufs=4) as w1pool,
        tc.tile_pool(name="stg", bufs=3) as stpool,
        tc.tile_pool(name="ppa", bufs=1, space="PSUM") as ppa,
        tc.tile_pool(name="ppt", bufs=1, space="PSUM") as ppt,
    ):
        bias0_sb = procst.tile([128, JW], f32)
        nc.sync.dma_start(bias0_sb[:], bias0d[:])

        for t in range(2):
            embt_sb = procst.tile([128, KC8, V], bf16, tag="embt")
            nc.sync.dma_start(embt_sb[:], embtd[:, t, :, :])
            rows_all = apool.tile([128, V // 128, ROWE], bf16, tag="a")
            for c in range(JW):
                cc = c % 4
                if c < JE:
                    if cc == 0:
                        w1c4 = w1pool.tile([128, 4, KC8, 128], bf16, tag="w1c")
                        nc.sync.dma_start(w1c4[:], w1d[t, c // 4])
                    lhs = w1c4[:, cc, :, :]
                else:
                    wgt = w1pool.tile([128, KC8, 128], bf16, tag="wgt")
                    nc.sync.dma_start(wgt[:], wgd[t])
                    lhs = wgt[:]
                psA = ppa.tile([128, V], f32, tag="pa")
                for kc in range(KC8):
                    nc.tensor.matmul(
                        psA[:],
                        lhs[:, kc, :],
                        embt_sb[:, kc, :],
                        start=(kc == 0),
                        stop=(kc == KC8 - 1),
                    )
                stage = stpool.tile([128, V], bf16, tag="stg")
                if t == 0:
                    nc.vector.tensor_scalar(
                        stage[:], psA[:], bias0_sb[:, c : c + 1], None, ALU.add
                    )
                else:
                    nc.vector.tensor_copy(stage[:], psA[:])
                psT = ppt.tile([128, V // 128, 128], bf16, tag="pt")
                for vc in range(V // 128):
                    nc.tensor.transpose(
                        psT[:, vc, :],
                        stage[:, vc * 128 : (vc + 1) * 128],
                        ident_sb[:],
                    )
                nc.scalar.copy(
                    rows_all[:, :, c * 128 : (c + 1) * 128], psT[:]
                )
            for vc in range(V // 128):
                nc.sync.dma_start(
                    atd[t * V + vc * 128 : t * V + (vc + 1) * 128, :],
                    rows_all[:, vc, :],
                )


def build_program(nst=NST, legalize=True):
    dt = mybir.dt
    f32, bf16 = dt.float32, dt.bfloat16
    AF = mybir.ActivationFunctionType
    ALU = mybir.AluOpType

    nc = bass.Bass()

    xd = nc.dram_tensor("xi01", [128, nst, 2 * ST // 16], dt.int16, kind="ExternalInput")
    embtd = nc.dram_tensor("embt", [128, 2, KC8, V], bf16, kind="ExternalInput")
    w1d = nc.dram_tensor("w1s", [2, JE // 4, 128, 4, KC8, 128], bf16, kind="ExternalInput")
    wgd = nc.dram_tensor("wgt", [2, 128, KC8, 128], bf16, kind="ExternalInput")
    w2d = nc.dram_tensor("w2s", [128, E, DC, OUT], bf16, kind="ExternalInput")
    b2d = nc.dram_tensor("b2s", [128, E], f32, kind="ExternalInput")
    bias0d = nc.dram_tensor("bias0", [128, JW], f32, kind="ExternalInput")
    identd = nc.dram_tensor("ident", [128, 128], bf16, kind="ExternalInput")
    seld = nc.dram_tensor("sels", [E, E, 128], bf16, kind="ExternalInput")
    outd = nc.dram_tensor("out", [128, nst * ST], f32, kind="ExternalOutput")

    # combined A table (both halves stacked), DRAM scratch
    atd = nc.dram_tensor("at01", [2 * V, ROWE], bf16, kind="Internal")

    with tile.TileContext(nc) as tc:
        with (
            tc.tile_pool(name="const", bufs=1) as cpool,
            tc.tile_pool(name="ga", bufs=3) as apool,
            tc.tile_pool(name="ht", bufs=2) as hpool,
            tc.tile_pool(name="sm", bufs=2) as smpool,
            tc.tile_pool(name="gsb", bufs=2) as gspool,
            tc.tile_pool(name="outp", bufs=2) as opool,
            tc.tile_pool(name="pzs", bufs=2, space="PSUM") as pzs,
            tc.tile_pool(name="peo", bufs=2, space="PSUM") as peo,
            tc.tile_pool(name="pgp", bufs=1, space="PSUM") as pgp,
            tc.tile_pool(name="pms", bufs=1, space="PSUM") as pms,
        ):
            from concourse import library_config

            nc.gpsimd.load_library(library_config.mlp)
            streg = nc.gpsimd.to_reg(2 * ST)

            # --- resident constants ---
            ident_sb = cpool.tile([128, 128], bf16)
            nc.sync.dma_start(ident_sb[:], identd[:])
            w2_sb = cpool.tile([128, E, DC, OUT], bf16)
            nc.sync.dma_start(w2_sb[:], w2d[:])
            b2_sb = cpool.tile([128, E], f32)
            nc.sync.dma_start(b2_sb[:], b2d[:])
            sel_sb = cpool.tile([E, E, 128], bf16)
            nc.sync.dma_start(sel_sb[:], seld[:])
            ones4 = cpool.tile([E, 1], bf16)
            nc.vector.memset(ones4[:], 1.0)
            ones14 = cpool.tile([1, E], bf16)
            nc.vector.memset(ones14[:], 1.0)

            # --- prologue: build A tables on the PE, write to DRAM scratch ---
            _build_tables(
                nc, tc, ALU, f32, bf16, embtd, bias0d, w1d, wgd, atd, ident_sb,
                apool,
            )

            # --- all gather indices resident up front (one DMA) ---
            xi_all = cpool.tile([128, nst, 2 * ST // 16], dt.int16, tag="xia")
            nc.sync.dma_start(xi_all[:], xd[:])

            # --- main loop: gather -> add -> transpose -> silu -> W2 -> mix ---
            def issue_gather(i):
                at_ = apool.tile([128, 2 * ST // 128, ROWE], bf16, tag="a")
                nc.gpsimd.dma_gather(
                    out_ap=at_[:],
                    in_ap=atd[:],
                    idxs_ap=xi_all[:, i, :],
                    num_idxs=2 * ST,
                    num_idxs_reg=streg,
                    elem_size=ROWE,
                    transpose=False,
                )
                return at_

            pend = [issue_gather(0)]
            if nst > 1:
                pend.append(issue_gather(1))

            NG = ST // 128
            for i in range(nst):
                at_ = pend.pop(0)
                for q in range(4):
                    c0 = q * 1024 if q < 3 else 3072
                    c1 = (q + 1) * 1024 if q < 3 else ROWE
                    nc.vector.tensor_tensor(
                        at_[:, 0:NG, c0:c1],
                        at_[:, 0:NG, c0:c1],
                        at_[:, NG : 2 * NG, c0:c1],
                        ALU.add,
                    )

                if i + 2 < nst:
                    pend.append(issue_gather(i + 2))

                h = hpool.tile([128, JE, ST], bf16, tag="h")
                for k in range(JE // 4):
                    zp = pzs.tile([128, 4, ST], bf16, tag="zp")
                    for j in range(4):
                        fc = 4 * k + j
                        for g in range(ST // 128):
                            nc.tensor.transpose(
                                zp[:, j, g * 128 : (g + 1) * 128],
                                at_[:, g, fc * 128 : (fc + 1) * 128],
                                ident_sb[:],
                            )
                    nc.scalar.activation(h[:, 4 * k : 4 * k + 4, :], zp[:], AF.Silu)

                # gating chunk (feature-major logits on partitions 0..3)
                pg = pzs.tile([128, 4, ST], bf16, tag="zp")
                for g in range(ST // 128):
                    nc.tensor.transpose(
                        pg[:, 0, g * 128 : (g + 1) * 128],
                        at_[:, g, JE * 128 : JE * 128 + 128],
                        ident_sb[:],
                    )
                expt = smpool.tile([E, ST], bf16, tag="expt")
                nc.scalar.activation(expt[:], pg[0:E, 0, :], AF.Exp)
                gsum = pms.tile([128, ST], f32, tag="gsum")
                sp = gsum[0:1, :]
                nc.tensor.matmul(sp, ones4[:], expt[:], start=True, stop=True)
                rec = smpool.tile([1, ST], f32, tag="rec")
                nc.vector.reciprocal_approx_fast(rec[:], sp)
                recb = smpool.tile([1, ST], bf16, tag="recb")
                nc.vector.tensor_copy(recb[:], rec[:])
                rb4 = gsum[32:36, :]
                nc.tensor.matmul(rb4, ones14[:], recb[:], start=True, stop=True)
                gates = smpool.tile([E, ST], bf16, tag="gates")
                nc.vector.tensor_tensor(gates[:], expt[:], rb4, ALU.mult)

                acc = opool.tile([128, ST], f32, tag="acc")
                for e in range(E):
                    eop = peo.tile([128, ST], f32, tag="eo")
                    for dc in range(DC):
                        nc.tensor.matmul(
                            eop[:],
                            w2_sb[:, e, dc, :],
                            h[:, e * DC + dc, :],
                            start=(dc == 0),
                            stop=(dc == DC - 1),
                        )
                    gp = pgp.tile([128, ST], f32, tag="gp")
                    nc.tensor.matmul(
                        gp[:], sel_sb[:, e, :], gates[:], start=True, stop=True
                    )
                    gpsb = gspool.tile([128, ST], bf16, tag="gpsb")
                    nc.vector.tensor_copy(gpsb[:], gp[:])
                    if e == 0:
                        nc.vector.scalar_tensor_tensor(
                            acc[:], eop[:], b2_sb[:, e : e + 1], gpsb[:],
                            ALU.add, ALU.mult,
                        )
                    else:
                        tmp = opool.tile([128, ST], f32, tag="tmp")
                        nc.vector.scalar_tensor_tensor(
                            tmp[:], eop[:], b2_sb[:, e : e + 1], gpsb[:],
                            ALU.add, ALU.mult,
                        )
                        nc.vector.tensor_add(acc[:], acc[:], tmp[:])
                nc.sync.dma_start(outd[:, i * ST : (i + 1) * ST], acc[:])

    if legalize:
        _legalize_waits(nc)
    mybir.codegen_inst_isa_subclasses(nc)
    return nc


def marshal_inputs(x, emb0, emb1, W1, b1, W2, b2, Wg, bg, nst=NST, ncores=NCORES):
    """Host-side: cast/reshape full inputs into per-core in_maps."""
    n_tok = ncores * nst * ST

    x = np.asarray(x)
    idx = np.concatenate(
        [
            x[:n_tok, 0].reshape(ncores, nst, ST),
            x[:n_tok, 1].reshape(ncores, nst, ST) + V,
        ],
        axis=2,
    ).astype(np.int16)                         # [c, nst, 2*ST]
    w = idx.reshape(ncores, nst, 2 * ST // 16, 16).transpose(0, 1, 3, 2)
    w = np.tile(w, (1, 1, 8, 1))               # [c, nst, 128, 2*ST//16]
    xh = {"xi01": np.ascontiguousarray(w.transpose(0, 2, 1, 3))}

    shared = {}
    # embt[p, t, kc, v] = emb_t[v, kc*128+p]  (partition-major, matches tile)
    embt = np.stack(
        [
            np.asarray(e).T.reshape(KC8, 128, V).transpose(1, 0, 2)
            for e in (emb0, emb1)
        ]
    ).transpose(1, 0, 2, 3)
    shared["embt"] = np.ascontiguousarray(embt.astype(BF16))

    # w1s[t, g, p, cc, kc, j]: chunk c = 4g+cc -> W1[e=c//8][t*1024+kc*128+p,
    # (c%8)*128+j]; gating tile wgt[t, p, kc, j] = Wg[t*1024+kc*128+p, j<4]
    W1r = np.asarray(W1).reshape(E, 2, KC8, 128, DC, 128)
    w1e = np.transpose(W1r, (1, 0, 4, 3, 2, 5)).reshape(2, JE, 128, KC8, 128)
    shared["w1s"] = np.ascontiguousarray(
        w1e.reshape(2, JE // 4, 4, 128, KC8, 128).transpose(0, 1, 3, 2, 4, 5)
        .astype(BF16)
    )
    wgt = np.zeros((2, 128, KC8, 128), dtype=np.float32)
    Wgr = np.asarray(Wg).reshape(2, KC8, 128, E)
    wgt[:, :, :, :E] = Wgr.transpose(0, 2, 1, 3)
    shared["wgt"] = np.ascontiguousarray(wgt.astype(BF16))

    shared["w2s"] = np.ascontiguousarray(
        np.asarray(W2).reshape(E, DC, 128, OUT).transpose(2, 0, 1, 3).astype(BF16)
    )
    shared["b2s"] = np.ascontiguousarray(np.asarray(b2).T.astype(np.float32))

    bias0 = np.zeros((128, JW), dtype=np.float32)
    bias0[:, :JE] = np.asarray(b1).reshape(E, DC, 128).transpose(2, 0, 1).reshape(128, JE)
    bias0[:E, JE] = np.asarray(bg)
    shared["bias0"] = np.ascontiguousarray(bias0)

    shared["ident"] = np.ascontiguousarray(np.eye(128, dtype=np.float32).astype(BF16))
    shared["sels"] = np.ascontiguousarray(
        np.broadcast_to(np.eye(E, dtype=np.float32)[:, :, None], (E, E, 128)).astype(
            BF16
        )
    )
    return [{**{k: v[c] for k, v in xh.items()}, **shared} for c in range(ncores)]


def kernel(x, emb0, emb1, W1, b1, W2, b2, Wg, bg):
    global LAST_EXEC_NS, LAST_RES
    nc = build_program()
    in_maps = marshal_inputs(x, emb0, emb1, W1, b1, W2, b2, Wg, bg)
    trace = os.environ.get("BASSMOE_TRACE", "0") == "1"
    res = run_bass_kernel_spmd(nc, in_maps, list(range(NCORES)), trace=trace)
    LAST_EXEC_NS = res.exec_time_ns
    LAST_RES = res
    out = np.empty((B, OUT), dtype=np.float32)
    for c in range(NCORES):
        out[c * BL : (c + 1) * BL, :] = res.results[c]["out"].T
    return out


# revision 26
# speedup vs baseline: 1.2215x; 1.1079x over previous
"""MoE model (embed -> gate -> 4 dense experts -> softmax combine) on 8 TRN2 cores.

Key algebraic restructuring vs the naive dense pipeline: the tokens only index
V=512 distinct embedding rows per table, so the entire first-layer expert
matmul (e @ W1, 8.4 MMAC/token — 95% of the model's FLOPs) is precomputed
per *vocab entry* instead of per token:

  A0[v] = concat_e(emb0[v] @ W1[e,:1024]) (+b1)   -> [V, E*D (+gating col)]
  A1[v] = concat_e(emb1[v] @ W1[e,1024:])         -> [V, E*D (+gating col)]
  z[t]  = A0[x0[t]] + A1[x1[t]]                   (gather + add)
  out[t]= sum_e softmax_e(z_gate) * (W2[e] @ silu(z[t,e,:]) + b2)

The gating logits (e @ Wg + bg) are folded into the same tables as a 33rd
128-wide column chunk, so one gather feeds both the experts and the gate.

Per core (8192 tokens): the A tables (4.2 MB each, bf16) are built on the PE
at kernel start (fm matmul + PE transpose) and written to DRAM scratch; the
main loop gathers token rows with *non-transposing* gpsimd gather DMAs
(token-major, 1 descriptor/row — descriptor-gen stays off the critical path),
adds them on the DVE, transposes z back to feature-major on the PE (128x128
identity matmuls into PSUM), applies Silu on the scalar engine straight out
of PSUM, and runs the small W2 matmul + softmax-weighted combine as before.

bf16 tables with fp32 PSUM accumulation: rel err vs fp32 reference ~0.5%.
"""

import os
import numpy as np
import ml_dtypes

import concourse.bass as bass
import concourse.mybir as mybir
import concourse.tile as tile
from concourse.bass_utils import run_bass_kernel_spmd

BF16 = ml_dtypes.bfloat16

B = 65536
V = 512
D = 1024
IN = 2048
E = 4
OUT = 128
NCORES = 8
BL = B // NCORES          # tokens per core
ST = 256                  # tokens per supertile
NST = BL // ST            # supertiles per core
KC8 = 8                   # 128-chunks of one table-half's input dim (1024)
JE = 32                   # expert-feature chunks per table row (E*D/128)
JW = JE + 1               # + 1 gating chunk
ROWE = JW * 128           # table row length in elements (4224)
DC = D // 128

LAST_EXEC_NS = None       # set when BASSMOE_TRACE=1
LAST_RES = None


def _legalize_waits(nc, max_waits=1):
    """This walrus build rejects instructions carrying more than ~1 sync-wait
    command ("Too many sync wait commands", CoreV2/V3GenImpl setupSyncWait).
    Hoist all but the last wait of every instruction onto single-wait NoOps
    placed immediately before it in the same engine's stream."""
    for f in nc.m.functions:
        for bb in f.blocks:
            insts = bb.instructions
            if not any(
                inst.sync_info is not None and len(inst.sync_info.on_wait) > max_waits
                for inst in insts
            ):
                continue
            new = []
            for inst in insts:
                si = inst.sync_info
                waits = list(si.on_wait) if si is not None else []
                if len(waits) > max_waits:
                    for w in waits[:-max_waits]:
                        nop = mybir.InstNoOp(
                            name=f"legw-{nc.next_id()}", ins=[], outs=[]
                        )
                        nop.engine = inst.engine
                        nop.sync_info = mybir.SyncInfo(on_wait=[w], on_update=[])
                        new.append(nop)
                    inst.sync_info = mybir.SyncInfo(
                        on_wait=waits[-max_waits:], on_update=list(si.on_update)
                    )
                new.append(inst)
            bb.instructions = new


def _build_tables(nc, tc, ALU, f32, bf16, embtd, bias0d, w1d, wgd, atd, ident_sb, apool):
    with (
        tc.tile_pool(name="procst", bufs=1) as procst,
        tc.tile_pool(name="w1st", bufs=2) as w1pool,
        tc.tile_pool(name="stg", bufs=2) as stpool,
        tc.tile_pool(name="ppa", bufs=2, space="PSUM") as ppa,
        tc.tile_pool(name="ppt", bufs=1, space="PSUM") as ppt,
    ):
        bias0_sb = procst.tile([128, JW], f32)
        nc.sync.dma_start(bias0_sb[:], bias0d[:])
        psT = ppt.tile([128, 4, 2, 128], bf16)

        for t in range(2):
            embt_sb = procst.tile([128, KC8, V], bf16, tag="embt")
            nc.sync.dma_start(embt_sb[:], embtd[:, t, :, :])
            rows_all = apool.tile([128, V // 128, ROWE], bf16, tag="a")
            stages = {}

            def emit_tr_copy(c):
                stage = stages.pop(c)
                sl = psT[:, :, c % 2, :]
                for vc in range(V // 128):
                    nc.tensor.transpose(
                        sl[:, vc, :],
                        stage[:, vc * 128 : (vc + 1) * 128],
                        ident_sb[:],
                    )
                nc.scalar.copy(rows_all[:, :, c * 128 : (c + 1) * 128], sl)

            for c in range(JW):
                cc = c % 4
                if c < JE:
                    if cc == 0:
                        w1c4 = w1pool.tile([128, 4, KC8, 128], bf16, tag="w1c")
                        nc.sync.dma_start(w1c4[:], w1d[t, c // 4])
                    lhs = w1c4[:, cc, :, :]
                else:
                    wgt = w1pool.tile([128, KC8, 128], bf16, tag="wgt", bufs=1)
                    nc.sync.dma_start(wgt[:], wgd[t])
                    lhs = wgt[:]
                psA = ppa.tile([128, V], f32, tag="pa")
                for kc in range(KC8):
                    nc.tensor.matmul(
                        psA[:],
                        lhs[:, kc, :],
                        embt_sb[:, kc, :],
                        start=(kc == 0),
                        stop=(kc == KC8 - 1),
                    )
                stage = stpool.tile([128, V], bf16, tag="stg")
                if t == 0:
                    nc.vector.tensor_scalar(
                        stage[:], psA[:], bias0_sb[:, c : c + 1], None, ALU.add
                    )
                else:
                    nc.vector.tensor_copy(stage[:], psA[:])
                stages[c] = stage
                if c > 0:
                    emit_tr_copy(c - 1)
            emit_tr_copy(JW - 1)
            for vc in range(V // 128):
                nc.sync.dma_start(
                    atd[t * V + vc * 128 : t * V + (vc + 1) * 128, :],
                    rows_all[:, vc, :],
                )


def build_program(nst=NST, legalize=True):
    dt = mybir.dt
    f32, bf16 = dt.float32, dt.bfloat16
    AF = mybir.ActivationFunctionType
    ALU = mybir.AluOpType

    nc = bass.Bass(dynamic_dma_scratch_size=32768)

    xd = nc.dram_tensor("xi01", [128, nst, 2 * ST // 16], dt.int16, kind="ExternalInput")
    embtd = nc.dram_tensor("embt", [128, 2, KC8, V], bf16, kind="ExternalInput")
    w1d = nc.dram_tensor("w1s", [2, JE // 4, 128, 4, KC8, 128], bf16, kind="ExternalInput")
    wgd = nc.dram_tensor("wgt", [2, 128, KC8, 128], bf16, kind="ExternalInput")
    w2d = nc.dram_tensor("w2s", [128, E, DC, OUT], bf16, kind="ExternalInput")
    b2d = nc.dram_tensor("b2s", [128, E], f32, kind="ExternalInput")
    bias0d = nc.dram_tensor("bias0", [128, JW], f32, kind="ExternalInput")
    identd = nc.dram_tensor("ident", [128, 128], bf16, kind="ExternalInput")
    seld = nc.dram_tensor("sels", [E, E, 128], bf16, kind="ExternalInput")
    outd = nc.dram_tensor("out", [128, nst * ST], f32, kind="ExternalOutput")

    # combined A table (both halves stacked), DRAM scratch
    atd = nc.dram_tensor("at01", [2 * V, ROWE], bf16, kind="Internal")

    with tile.TileContext(nc) as tc:
        with (
            tc.tile_pool(name="const", bufs=1) as cpool,
            tc.tile_pool(name="ga", bufs=3) as apool,
            tc.tile_pool(name="ht", bufs=2) as hpool,
            tc.tile_pool(name="sm", bufs=2) as smpool,
            tc.tile_pool(name="gsb", bufs=2) as gspool,
            tc.tile_pool(name="outp", bufs=2) as opool,
            tc.tile_pool(name="pzs", bufs=2, space="PSUM") as pzs,
            tc.tile_pool(name="peo", bufs=2, space="PSUM") as peo,
            tc.tile_pool(name="pgx", bufs=1, space="PSUM") as pgx,
        ):
            from concourse import library_config

            nc.gpsimd.load_library(library_config.mlp)
            streg = nc.gpsimd.to_reg(2 * ST)

            # --- resident constants ---
            ident_sb = cpool.tile([128, 128], bf16)
            nc.sync.dma_start(ident_sb[:], identd[:])
            w2_sb = cpool.tile([128, E, DC, OUT], bf16)
            nc.sync.dma_start(w2_sb[:], w2d[:])
            b2_sb = cpool.tile([128, E], f32)
            nc.sync.dma_start(b2_sb[:], b2d[:])
            sel_sb = cpool.tile([E, E, 128], bf16)
            nc.sync.dma_start(sel_sb[:], seld[:])
            ones4 = cpool.tile([E, 1], bf16)
            nc.vector.memset(ones4[:], 1.0)
            ones14 = cpool.tile([1, E], bf16)
            nc.vector.memset(ones14[:], 1.0)

            # --- prologue: build A tables on the PE, write to DRAM scratch ---
            _build_tables(
                nc, tc, ALU, f32, bf16, embtd, bias0d, w1d, wgd, atd, ident_sb,
                apool,
            )

            # --- all gather indices resident up front (one DMA) ---
            xi_all = cpool.tile([128, nst, 2 * ST // 16], dt.int16, tag="xia")
            nc.sync.dma_start(xi_all[:], xd[:])

            # --- main loop: gather -> add -> transpose -> silu -> W2 -> mix ---
            def issue_gather(i):
                at_ = apool.tile([128, 2 * ST // 128, ROWE], bf16, tag="a")
                nc.gpsimd.dma_gather(
                    out_ap=at_[:],
                    in_ap=atd[:],
                    idxs_ap=xi_all[:, i, :],
                    num_idxs=2 * ST,
                    num_idxs_reg=streg,
                    elem_size=ROWE,
                    transpose=False,
                )
                return at_

            pend = [issue_gather(0)]
            if nst > 1:
                pend.append(issue_gather(1))

            NG = ST // 128
            for i in range(nst):
                at_ = pend.pop(0)
                for q in range(4):
                    c0 = q * 1024 if q < 3 else 3072
                    c1 = (q + 1) * 1024 if q < 3 else ROWE
                    nc.vector.tensor_tensor(
                        at_[:, 0:NG, c0:c1],
                        at_[:, 0:NG, c0:c1],
                        at_[:, NG : 2 * NG, c0:c1],
                        ALU.add,
                    )

                if i + 2 < nst:
                    pend.append(issue_gather(i + 2))

                h = hpool.tile([128, JE, ST], bf16, tag="h")
                for k in range(JE // 4):
                    zp = pzs.tile([128, 4, ST], bf16, tag="zp")
                    for j in range(4):
                        fc = 4 * k + j
                        for g in range(ST // 128):
                            nc.tensor.transpose(
                                zp[:, j, g * 128 : (g + 1) * 128],
                                at_[:, g, fc * 128 : (fc + 1) * 128],
                                ident_sb[:],
                            )
                    nc.scalar.activation(h[:, 4 * k : 4 * k + 4, :], zp[:], AF.Silu)

                # gating chunk (feature-major logits on partitions 0..3)
                pg = pzs.tile([128, 4, ST], bf16, tag="zp")
                for g in range(ST // 128):
                    nc.tensor.transpose(
                        pg[:, 0, g * 128 : (g + 1) * 128],
                        at_[:, g, JE * 128 : JE * 128 + 128],
                        ident_sb[:],
                    )
                expt = smpool.tile([E, ST], bf16, tag="expt")
                nc.scalar.activation(expt[:], pg[0:E, 0, :], AF.Exp)
                gx = pgx.tile([128, 2, ST], f32, tag="gx")
                sp = gx[0:1, 1, :]
                nc.tensor.matmul(sp, ones4[:], expt[:], start=True, stop=True)
                rec = smpool.tile([1, ST], f32, tag="rec", bufs=1)
                nc.vector.reciprocal_approx_fast(rec[:], sp)
                recb = smpool.tile([1, ST], bf16, tag="recb")
                nc.vector.tensor_copy(recb[:], rec[:])
                rb4 = gx[32:36, 1, :]
                nc.tensor.matmul(rb4, ones14[:], recb[:], start=True, stop=True)
                gates = smpool.tile([E, ST], bf16, tag="gates")
                nc.vector.tensor_tensor(gates[:], expt[:], rb4, ALU.mult)

                acc = opool.tile([128, ST], f32, tag="acc")
                for e in range(E):
                    eop = peo.tile([128, ST], f32, tag="eo")
                    for dc in range(DC):
                        nc.tensor.matmul(
                            eop[:],
                            w2_sb[:, e, dc, :],
                            h[:, e * DC + dc, :],
                            start=(dc == 0),
                            stop=(dc == DC - 1),
                        )
                    gp = gx[:, 0, :]
                    nc.tensor.matmul(
                        gp, sel_sb[:, e, :], gates[:], start=True, stop=True
                    )
                    gp = gx[:, 0, :]
                    gpsb = gspool.tile([128, ST], bf16, tag="gpsb")
                    nc.vector.tensor_copy(gpsb[:], gp)
                    if e == 0:
                        nc.vector.scalar_tensor_tensor(
                            acc[:], eop[:], b2_sb[:, e : e + 1], gpsb[:],
                            ALU.add, ALU.mult,
                        )
                    else:
                        tmp = opool.tile([128, ST], f32, tag="tmp", bufs=1)
                        nc.vector.scalar_tensor_tensor(
                            tmp[:], eop[:], b2_sb[:, e : e + 1], gpsb[:],
                            ALU.add, ALU.mult,
                        )
                        nc.vector.tensor_add(acc[:], acc[:], tmp[:])
                nc.sync.dma_start(outd[:, i * ST : (i + 1) * ST], acc[:])

    if legalize:
        _legalize_waits(nc)
    mybir.codegen_inst_isa_subclasses(nc)
    return nc


def marshal_inputs(x, emb0, emb1, W1, b1, W2, b2, Wg, bg, nst=NST, ncores=NCORES):
    """Host-side: cast/reshape full inputs into per-core in_maps."""
    n_tok = ncores * nst * ST

    x = np.asarray(x)
    idx = np.concatenate(
        [
            x[:n_tok, 0].reshape(ncores, nst, ST),
            x[:n_tok, 1].reshape(ncores, nst, ST) + V,
        ],
        axis=2,
    ).astype(np.int16)                         # [c, nst, 2*ST]
    w = idx.reshape(ncores, nst, 2 * ST // 16, 16).transpose(0, 1, 3, 2)
    w = np.tile(w, (1, 1, 8, 1))               # [c, nst, 128, 2*ST//16]
    xh = {"xi01": np.ascontiguousarray(w.transpose(0, 2, 1, 3))}

    shared = {}
    # embt[p, t, kc, v] = emb_t[v, kc*128+p]  (partition-major, matches tile)
    embt = np.stack(
        [
            np.asarray(e).T.reshape(KC8, 128, V).transpose(1, 0, 2)
            for e in (emb0, emb1)
        ]
    ).transpose(1, 0, 2, 3)
    shared["embt"] = np.ascontiguousarray(embt.astype(BF16))

    # w1s[t, g, p, cc, kc, j]: chunk c = 4g+cc -> W1[e=c//8][t*1024+kc*128+p,
    # (c%8)*128+j]; gating tile wgt[t, p, kc, j] = Wg[t*1024+kc*128+p, j<4]
    W1r = np.asarray(W1).reshape(E, 2, KC8, 128, DC, 128)
    w1e = np.transpose(W1r, (1, 0, 4, 3, 2, 5)).reshape(2, JE, 128, KC8, 128)
    shared["w1s"] = np.ascontiguousarray(
        w1e.reshape(2, JE // 4, 4, 128, KC8, 128).transpose(0, 1, 3, 2, 4, 5)
        .astype(BF16)
    )
    wgt = np.zeros((2, 128, KC8, 128), dtype=np.float32)
    Wgr = np.asarray(Wg).reshape(2, KC8, 128, E)
    wgt[:, :, :, :E] = Wgr.transpose(0, 2, 1, 3)
    shared["wgt"] = np.ascontiguousarray(wgt.astype(BF16))

    shared["w2s"] = np.ascontiguousarray(
        np.asarray(W2).reshape(E, DC, 128, OUT).transpose(2, 0, 1, 3).astype(BF16)
    )
    shared["b2s"] = np.ascontiguousarray(np.asarray(b2).T.astype(np.float32))

    bias0 = np.zeros((128, JW), dtype=np.float32)
    bias0[:, :JE] = np.asarray(b1).reshape(E, DC, 128).transpose(2, 0, 1).reshape(128, JE)
    bias0[:E, JE] = np.asarray(bg)
    shared["bias0"] = np.ascontiguousarray(bias0)

    shared["ident"] = np.ascontiguousarray(np.eye(128, dtype=np.float32).astype(BF16))
    shared["sels"] = np.ascontiguousarray(
        np.broadcast_to(np.eye(E, dtype=np.float32)[:, :, None], (E, E, 128)).astype(
            BF16
        )
    )
    return [{**{k: v[c] for k, v in xh.items()}, **shared} for c in range(ncores)]


def kernel(x, emb0, emb1, W1, b1, W2, b2, Wg, bg):
    global LAST_EXEC_NS, LAST_RES
    nc = build_program()
    in_maps = marshal_inputs(x, emb0, emb1, W1, b1, W2, b2, Wg, bg)
    trace = os.environ.get("BASSMOE_TRACE", "0") == "1"
    res = run_bass_kernel_spmd(nc, in_maps, list(range(NCORES)), trace=trace)
    LAST_EXEC_NS = res.exec_time_ns
    LAST_RES = res
    out = np.empty((B, OUT), dtype=np.float32)
    for c in range(NCORES):
        out[c * BL : (c + 1) * BL, :] = res.results[c]["out"].T
    return out


# revision 27
# speedup vs baseline: 1.2234x; 1.0015x over previous
"""MoE model (embed -> gate -> 4 dense experts -> softmax combine) on 8 TRN2 cores.

Key algebraic restructuring vs the naive dense pipeline: the tokens only index
V=512 distinct embedding rows per table, so the entire first-layer expert
matmul (e @ W1, 8.4 MMAC/token — 95% of the model's FLOPs) is precomputed
per *vocab entry* instead of per token:

  A0[v] = concat_e(emb0[v] @ W1[e,:1024]) (+b1)   -> [V, E*D (+gating col)]
  A1[v] = concat_e(emb1[v] @ W1[e,1024:])         -> [V, E*D (+gating col)]
  z[t]  = A0[x0[t]] + A1[x1[t]]                   (gather + add)
  out[t]= sum_e softmax_e(z_gate) * (W2[e] @ silu(z[t,e,:]) + b2)

The gating logits (e @ Wg + bg) are folded into the same tables as a 33rd
128-wide column chunk, so one gather feeds both the experts and the gate.

Per core (8192 tokens): the A tables (4.2 MB each, bf16) are built on the PE
at kernel start (fm matmul + PE transpose) and written to DRAM scratch; the
main loop gathers token rows with *non-transposing* gpsimd gather DMAs
(token-major, 1 descriptor/row — descriptor-gen stays off the critical path),
adds them on the DVE, transposes z back to feature-major on the PE (128x128
identity matmuls into PSUM), applies Silu on the scalar engine straight out
of PSUM, and runs the small W2 matmul + softmax-weighted combine as before.

bf16 tables with fp32 PSUM accumulation: rel err vs fp32 reference ~0.5%.
"""

import os
import numpy as np
import ml_dtypes

import concourse.bass as bass
import concourse.mybir as mybir
import concourse.tile as tile
from concourse.bass_utils import run_bass_kernel_spmd

BF16 = ml_dtypes.bfloat16

B = 65536
V = 512
D = 1024
IN = 2048
E = 4
OUT = 128
NCORES = 8
BL = B // NCORES          # tokens per core
ST = 256                  # tokens per supertile
NST = BL // ST            # supertiles per core
KC8 = 8                   # 128-chunks of one table-half's input dim (1024)
JE = 32                   # expert-feature chunks per table row (E*D/128)
JW = JE + 1               # + 1 gating chunk
ROWE = JW * 128           # table row length in elements (4224)
DC = D // 128

LAST_EXEC_NS = None       # set when BASSMOE_TRACE=1
LAST_RES = None


def _legalize_waits(nc, max_waits=1):
    """This walrus build rejects instructions carrying more than ~1 sync-wait
    command ("Too many sync wait commands", CoreV2/V3GenImpl setupSyncWait).
    Hoist all but the last wait of every instruction onto single-wait NoOps
    placed immediately before it in the same engine's stream."""
    for f in nc.m.functions:
        for bb in f.blocks:
            insts = bb.instructions
            if not any(
                inst.sync_info is not None and len(inst.sync_info.on_wait) > max_waits
                for inst in insts
            ):
                continue
            new = []
            for inst in insts:
                si = inst.sync_info
                waits = list(si.on_wait) if si is not None else []
                if len(waits) > max_waits:
                    for w in waits[:-max_waits]:
                        nop = mybir.InstNoOp(
                            name=f"legw-{nc.next_id()}", ins=[], outs=[]
                        )
                        nop.engine = inst.engine
                        nop.sync_info = mybir.SyncInfo(on_wait=[w], on_update=[])
                        new.append(nop)
                    inst.sync_info = mybir.SyncInfo(
                        on_wait=waits[-max_waits:], on_update=list(si.on_update)
                    )
                new.append(inst)
            bb.instructions = new


def _build_tables(nc, tc, ALU, f32, bf16, embtd, bias0d, w1d, wgd, atd, ident_sb, apool):
    with (
        tc.tile_pool(name="procst", bufs=1) as procst,
        tc.tile_pool(name="w1st", bufs=2) as w1pool,
        tc.tile_pool(name="stg", bufs=2) as stpool,
        tc.tile_pool(name="ppa", bufs=2, space="PSUM") as ppa,
        tc.tile_pool(name="ppt", bufs=1, space="PSUM") as ppt,
    ):
        bias0_sb = procst.tile([128, JW], f32)
        nc.sync.dma_start(bias0_sb[:], bias0d[:])
        psT = ppt.tile([128, 4, 2, 128], bf16)

        for t in range(2):
            embt_sb = procst.tile([128, KC8, V], bf16, tag="embt")
            nc.sync.dma_start(embt_sb[:], embtd[:, t, :, :])
            rows_all = apool.tile([128, V // 128, ROWE], bf16, tag="a")
            stages = {}

            def emit_tr_copy(c):
                stage = stages.pop(c)
                sl = psT[:, :, c % 2, :]
                for vc in range(V // 128):
                    nc.tensor.transpose(
                        sl[:, vc, :],
                        stage[:, vc * 128 : (vc + 1) * 128],
                        ident_sb[:],
                    )
                nc.scalar.copy(rows_all[:, :, c * 128 : (c + 1) * 128], sl)

            for c in range(JW):
                cc = c % 4
                if c < JE:
                    if cc == 0:
                        w1c4 = w1pool.tile([128, 4, KC8, 128], bf16, tag="w1c")
                        nc.sync.dma_start(w1c4[:], w1d[t, c // 4])
                    lhs = w1c4[:, cc, :, :]
                else:
                    wgt = w1pool.tile([128, KC8, 128], bf16, tag="wgt", bufs=1)
                    nc.sync.dma_start(wgt[:], wgd[t])
                    lhs = wgt[:]
                psA = ppa.tile([128, V], f32, tag="pa")
                for kc in range(KC8):
                    nc.tensor.matmul(
                        psA[:],
                        lhs[:, kc, :],
                        embt_sb[:, kc, :],
                        start=(kc == 0),
                        stop=(kc == KC8 - 1),
                    )
                stage = stpool.tile([128, V], bf16, tag="stg")
                if t == 0:
                    nc.vector.tensor_scalar(
                        stage[:], psA[:], bias0_sb[:, c : c + 1], None, ALU.add
                    )
                else:
                    nc.vector.tensor_copy(stage[:], psA[:])
                stages[c] = stage
                if c > 0:
                    emit_tr_copy(c - 1)
            emit_tr_copy(JW - 1)
            for vc in range(V // 128):
                nc.sync.dma_start(
                    atd[t * V + vc * 128 : t * V + (vc + 1) * 128, :],
                    rows_all[:, vc, :],
                )


def build_program(nst=NST, legalize=True):
    dt = mybir.dt
    f32, bf16 = dt.float32, dt.bfloat16
    AF = mybir.ActivationFunctionType
    ALU = mybir.AluOpType

    nc = bass.Bass(dynamic_dma_scratch_size=32768, num_swdge_queues=2)

    xd = nc.dram_tensor("xi01", [128, nst, 2 * ST // 16], dt.int16, kind="ExternalInput")
    embtd = nc.dram_tensor("embt", [128, 2, KC8, V], bf16, kind="ExternalInput")
    w1d = nc.dram_tensor("w1s", [2, JE // 4, 128, 4, KC8, 128], bf16, kind="ExternalInput")
    wgd = nc.dram_tensor("wgt", [2, 128, KC8, 128], bf16, kind="ExternalInput")
    w2d = nc.dram_tensor("w2s", [128, E, DC, OUT], bf16, kind="ExternalInput")
    b2d = nc.dram_tensor("b2s", [128, E], f32, kind="ExternalInput")
    bias0d = nc.dram_tensor("bias0", [128, JW], f32, kind="ExternalInput")
    identd = nc.dram_tensor("ident", [128, 128], bf16, kind="ExternalInput")
    seld = nc.dram_tensor("sels", [E, E, 128], bf16, kind="ExternalInput")
    outd = nc.dram_tensor("out", [128, nst * ST], f32, kind="ExternalOutput")

    # combined A table (both halves stacked), DRAM scratch
    atd = nc.dram_tensor("at01", [2 * V, ROWE], bf16, kind="Internal")

    with tile.TileContext(nc) as tc:
        with (
            tc.tile_pool(name="const", bufs=1) as cpool,
            tc.tile_pool(name="ga", bufs=3) as apool,
            tc.tile_pool(name="ht", bufs=2) as hpool,
            tc.tile_pool(name="sm", bufs=2) as smpool,
            tc.tile_pool(name="gsb", bufs=2) as gspool,
            tc.tile_pool(name="outp", bufs=2) as opool,
            tc.tile_pool(name="pzs", bufs=2, space="PSUM") as pzs,
            tc.tile_pool(name="peo", bufs=2, space="PSUM") as peo,
            tc.tile_pool(name="pgx", bufs=1, space="PSUM") as pgx,
        ):
            from concourse import library_config

            nc.gpsimd.load_library(library_config.mlp)
            streg = nc.gpsimd.to_reg(2 * ST)

            # --- resident constants ---
            ident_sb = cpool.tile([128, 128], bf16)
            nc.sync.dma_start(ident_sb[:], identd[:])
            w2_sb = cpool.tile([128, E, DC, OUT], bf16)
            nc.sync.dma_start(w2_sb[:], w2d[:])
            b2_sb = cpool.tile([128, E], f32)
            nc.sync.dma_start(b2_sb[:], b2d[:])
            sel_sb = cpool.tile([E, E, 128], bf16)
            nc.sync.dma_start(sel_sb[:], seld[:])
            ones4 = cpool.tile([E, 1], bf16)
            nc.vector.memset(ones4[:], 1.0)
            ones14 = cpool.tile([1, E], bf16)
            nc.vector.memset(ones14[:], 1.0)

            # --- prologue: build A tables on the PE, write to DRAM scratch ---
            _build_tables(
                nc, tc, ALU, f32, bf16, embtd, bias0d, w1d, wgd, atd, ident_sb,
                apool,
            )

            # --- all gather indices resident up front (one DMA) ---
            xi_all = cpool.tile([128, nst, 2 * ST // 16], dt.int16, tag="xia")
            nc.sync.dma_start(xi_all[:], xd[:])

            # --- main loop: gather -> add -> transpose -> silu -> W2 -> mix ---
            def issue_gather(i):
                at_ = apool.tile([128, 2 * ST // 128, ROWE], bf16, tag="a")
                nc.gpsimd.dma_gather(
                    out_ap=at_[:],
                    in_ap=atd[:],
                    idxs_ap=xi_all[:, i, :],
                    num_idxs=2 * ST,
                    num_idxs_reg=streg,
                    elem_size=ROWE,
                    transpose=False,
                    queue_num=i % 2,
                )
                return at_

            pend = [issue_gather(0)]
            if nst > 1:
                pend.append(issue_gather(1))

            NG = ST // 128
            for i in range(nst):
                at_ = pend.pop(0)
                for q in range(4):
                    c0 = q * 1024 if q < 3 else 3072
                    c1 = (q + 1) * 1024 if q < 3 else ROWE
                    nc.vector.tensor_tensor(
                        at_[:, 0:NG, c0:c1],
                        at_[:, 0:NG, c0:c1],
                        at_[:, NG : 2 * NG, c0:c1],
                        ALU.add,
                    )

                if i + 2 < nst:
                    pend.append(issue_gather(i + 2))

                h = hpool.tile([128, JE, ST], bf16, tag="h")
                for k in range(JE // 4):
                    zp = pzs.tile([128, 4, ST], bf16, tag="zp")
                    for j in range(4):
                        fc = 4 * k + j
                        for g in range(ST // 128):
                            nc.tensor.transpose(
                                zp[:, j, g * 128 : (g + 1) * 128],
                                at_[:, g, fc * 128 : (fc + 1) * 128],
                                ident_sb[:],
                            )
                    nc.scalar.activation(h[:, 4 * k : 4 * k + 4, :], zp[:], AF.Silu)

                # gating chunk (feature-major logits on partitions 0..3)
                pg = pzs.tile([128, 4, ST], bf16, tag="zp")
                for g in range(ST // 128):
                    nc.tensor.transpose(
                        pg[:, 0, g * 128 : (g + 1) * 128],
                        at_[:, g, JE * 128 : JE * 128 + 128],
                        ident_sb[:],
                    )
                expt = smpool.tile([E, ST], bf16, tag="expt")
                nc.scalar.activation(expt[:], pg[0:E, 0, :], AF.Exp)
                gx = pgx.tile([128, 2, ST], f32, tag="gx")
                sp = gx[0:1, 1, :]
                nc.tensor.matmul(sp, ones4[:], expt[:], start=True, stop=True)
                rec = smpool.tile([1, ST], f32, tag="rec", bufs=1)
                nc.vector.reciprocal_approx_fast(rec[:], sp)
                recb = smpool.tile([1, ST], bf16, tag="recb")
                nc.vector.tensor_copy(recb[:], rec[:])
                rb4 = gx[32:36, 1, :]
                nc.tensor.matmul(rb4, ones14[:], recb[:], start=True, stop=True)
                gates = smpool.tile([E, ST], bf16, tag="gates")
                nc.vector.tensor_tensor(gates[:], expt[:], rb4, ALU.mult)

                acc = opool.tile([128, ST], f32, tag="acc")
                for e in range(E):
                    eop = peo.tile([128, ST], f32, tag="eo")
                    for dc in range(DC):
                        nc.tensor.matmul(
                            eop[:],
                            w2_sb[:, e, dc, :],
                            h[:, e * DC + dc, :],
                            start=(dc == 0),
                            stop=(dc == DC - 1),
                        )
                    gp = gx[:, 0, :]
                    nc.tensor.matmul(
                        gp, sel_sb[:, e, :], gates[:], start=True, stop=True
                    )
                    gp = gx[:, 0, :]
                    gpsb = gspool.tile([128, ST], bf16, tag="gpsb")
                    nc.vector.tensor_copy(gpsb[:], gp)
                    if e == 0:
                        nc.vector.scalar_tensor_tensor(
                            acc[:], eop[:], b2_sb[:, e : e + 1], gpsb[:],
                            ALU.add, ALU.mult,
                        )
                    else:
                        tmp = opool.tile([128, ST], f32, tag="tmp", bufs=1)
                        nc.vector.scalar_tensor_tensor(
                            tmp[:], eop[:], b2_sb[:, e : e + 1], gpsb[:],
                            ALU.add, ALU.mult,
                        )
                        nc.vector.tensor_add(acc[:], acc[:], tmp[:])
                nc.sync.dma_start(outd[:, i * ST : (i + 1) * ST], acc[:])

    if legalize:
        _legalize_waits(nc)
    mybir.codegen_inst_isa_subclasses(nc)
    return nc


def marshal_inputs(x, emb0, emb1, W1, b1, W2, b2, Wg, bg, nst=NST, ncores=NCORES):
    """Host-side: cast/reshape full inputs into per-core in_maps."""
    n_tok = ncores * nst * ST

    x = np.asarray(x)
    idx = np.concatenate(
        [
            x[:n_tok, 0].reshape(ncores, nst, ST),
            x[:n_tok, 1].reshape(ncores, nst, ST) + V,
        ],
        axis=2,
    ).astype(np.int16)                         # [c, nst, 2*ST]
    w = idx.reshape(ncores, nst, 2 * ST // 16, 16).transpose(0, 1, 3, 2)
    w = np.tile(w, (1, 1, 8, 1))               # [c, nst, 128, 2*ST//16]
    xh = {"xi01": np.ascontiguousarray(w.transpose(0, 2, 1, 3))}

    shared = {}
    # embt[p, t, kc, v] = emb_t[v, kc*128+p]  (partition-major, matches tile)
    embt = np.stack(
        [
            np.asarray(e).T.reshape(KC8, 128, V).transpose(1, 0, 2)
            for e in (emb0, emb1)
        ]
    ).transpose(1, 0, 2, 3)
    shared["embt"] = np.ascontiguousarray(embt.astype(BF16))

    # w1s[t, g, p, cc, kc, j]: chunk c = 4g+cc -> W1[e=c//8][t*1024+kc*128+p,
    # (c%8)*128+j]; gating tile wgt[t, p, kc, j] = Wg[t*1024+kc*128+p, j<4]
    W1r = np.asarray(W1).reshape(E, 2, KC8, 128, DC, 128)
    w1e = np.transpose(W1r, (1, 0, 4, 3, 2, 5)).reshape(2, JE, 128, KC8, 128)
    shared["w1s"] = np.ascontiguousarray(
        w1e.reshape(2, JE // 4, 4, 128, KC8, 128).transpose(0, 1, 3, 2, 4, 5)
        .astype(BF16)
    )
    wgt = np.zeros((2, 128, KC8, 128), dtype=np.float32)
    Wgr = np.asarray(Wg).reshape(2, KC8, 128, E)
    wgt[:, :, :, :E] = Wgr.transpose(0, 2, 1, 3)
    shared["wgt"] = np.ascontiguousarray(wgt.astype(BF16))

    shared["w2s"] = np.ascontiguousarray(
        np.asarray(W2).reshape(E, DC, 128, OUT).transpose(2, 0, 1, 3).astype(BF16)
    )
    shared["b2s"] = np.ascontiguousarray(np.asarray(b2).T.astype(np.float32))

    bias0 = np.zeros((128, JW), dtype=np.float32)
    bias0[:, :JE] = np.asarray(b1).reshape(E, DC, 128).transpose(2, 0, 1).reshape(128, JE)
    bias0[:E, JE] = np.asarray(bg)
    shared["bias0"] = np.ascontiguousarray(bias0)

    shared["ident"] = np.ascontiguousarray(np.eye(128, dtype=np.float32).astype(BF16))
    shared["sels"] = np.ascontiguousarray(
        np.broadcast_to(np.eye(E, dtype=np.float32)[:, :, None], (E, E, 128)).astype(
            BF16
        )
    )
    return [{**{k: v[c] for k, v in xh.items()}, **shared} for c in range(ncores)]


def kernel(x, emb0, emb1, W1, b1, W2, b2, Wg, bg):
    global LAST_EXEC_NS, LAST_RES
    nc = build_program()
    in_maps = marshal_inputs(x, emb0, emb1, W1, b1, W2, b2, Wg, bg)
    trace = os.environ.get("BASSMOE_TRACE", "0") == "1"
    res = run_bass_kernel_spmd(nc, in_maps, list(range(NCORES)), trace=trace)
    LAST_EXEC_NS = res.exec_time_ns
    LAST_RES = res
    out = np.empty((B, OUT), dtype=np.float32)
    for c in range(NCORES):
        out[c * BL : (c + 1) * BL, :] = res.results[c]["out"].T
    return out
